# revision 32
# baseline (speedup 1.0000x reference)
"""Trainium2 Bass kernel for nn_DotProductAttention_11433202942822.

Math (per (b, h) pair, T=2048, D=64):
    S = Q @ K^T * (1/sqrt(64))            [T1, T2]
    attn = softmax(S, axis=T1)            <- softmax over the QUERY axis
    out = attn @ V                        [T1, D]

Restructuring (as in the previous version):
  * Compute S^T = K @ Q^T with k2 on partitions and q on the free axis, so
    the softmax reduction (over q) is a free-axis reduction.
  * Fold the softmax normalization into V (vp = V / s), scaling 2048x64
    elements instead of 2048x2048.
  * Matmuls in fp16, N=512 moving chunks, row-group (mm1) and col-group
    (mm2) pairing for PE-array concurrency.

New in this version — the exp work (16.7M elements/core, previously all on
ScalarE at 1 elem/cycle/lane => ~150us busy) is SPLIT between ScalarE and
VectorE:
  * ScalarE chunks: activation(Exp, accum_out) as before.
  * VectorE chunks: a custom DVE op computes a Schraudolph-style exp
    directly to fp16 BITS: i16 = x*A + 15360 - (frac*c)^2 written through
    an int16-bitcast AP; the int16 bit pattern IS the fp16 exp value.
    A quadratic mantissa-phase correction keeps the per-element rel error
    ~0.9% std (multiplicative bias cancels exactly in the softmax ratio).
    Column sums for those chunks come from one scalar_tensor_tensor
    (halves-add with accum_out) on the fp16 values.
  * Per-tile normalization (sum-merge + V/s) moves to the idle GPSIMD.

Sharding: batch*heads = 32 pairs, 4 per core across 8 cores (head/data
parallel, no cross-core communication).
"""

import sys

import numpy as np

if "/opt/trn_rl_repo" not in sys.path:
    sys.path.insert(0, "/opt/trn_rl_repo")

import concourse.tile as tile  # noqa: E402
from concourse import bacc, mybir  # noqa: E402
from concourse.bass_utils import run_bass_kernel_spmd  # noqa: E402


# note: walrus's --enable-ldw-opt=true was tried and is rejected by this
# toolchain ("InstLdweights is not compatible with LDW optimization").

P = 128
D = 64
SCALE = 1.0 / (D ** 0.5)
N_CORES = 8

F32 = mybir.dt.float32
F16 = mybir.dt.float16
I16 = mybir.dt.int16

# ---- Schraudolph fp16 exp constants (see numpy calibration) ---------------
SCH_A = float(np.float32(1024.0 / np.log(2.0) / 8.0))  # x_raw * SCALE * 1024/ln2
SCH_B = 15360.0                                        # 15 * 1024 (fp16 bias<<10)
SCH_M = float(np.float32(1.5 * 2.0 ** 33))             # magic: rounds u to mult of 1024
SCH_C3 = 0.01720                                       # mantissa-phase parabola scale

# B-tile chunks (pair j, chunk c) handed to ScalarE for load balance.
ACT_EXTRA = frozenset({(1, 1), (3, 0), (5, 1), (7, 0)})

_SCHRAUD_OP = None


def _get_schraud_op():
    """Register the custom DVE op (idempotent) and return it."""
    global _SCHRAUD_OP
    if _SCHRAUD_OP is not None:
        return _SCHRAUD_OP
    from concourse import dve_ops as dvo
    from concourse.dve_spec import (
        C0, C1, C2, C3, Spec, Src0, _has_src1, _spill_c3_to_src1, lower, sq,
    )
    from concourse.dve_uop import DveOpSpec

    name = "SCHRAUD_EXP16_ANT"
    if name in dvo._SUB_OPCODE_FOR_NAME:
        _SCHRAUD_OP = next(o for o in dvo.OPS if o.name == name)
        return _SCHRAUD_OP

    _u0 = Src0 * C0 + C1
    _r = (_u0 + C2) - C2
    _f = _u0 - _r
    _q = sq(_f * C3)
    body = _spill_c3_to_src1(_u0 - _q)

    def _ref(in0, in1, s0, s1, imm2):
        c3 = np.asarray(in1, np.float32).reshape(-1, 1)
        x = in0.astype(np.float32)
        u0 = (x * np.float32(s0) + np.float32(s1)).astype(np.float32)
        r = ((u0 + np.float32(imm2)).astype(np.float32)
             - np.float32(imm2)).astype(np.float32)
        f = (u0 - r).astype(np.float32)
        q = ((f * c3).astype(np.float32) ** 2).astype(np.float32)
        return (u0 - q).astype(np.float32)

    spec = Spec(body=body, reference=_ref)
    row = dvo._CUSTOM_DVE_ROW_BASE + len(dvo.OPS)
    dvo._SUB_OPCODE_FOR_NAME[name] = row
    shas = {}
    for ver in ("v3", "v4"):
        ospec = DveOpSpec(name=name, opcode=row, uops=lower(spec, ver=ver),
                          rd1_en=_has_src1(spec))
        shas[ver] = ospec.sha(ver)
        dvo._COMPILE_CACHE[(name, ver)] = ospec
    op = dvo.DveOp(name, spec, subdim=False, uops_sha=shas)
    dvo.OPS.append(op)
    dvo.CUSTOM_DVE_SPECS[name] = spec
    _SCHRAUD_OP = op
    return op


def build_attention_nc(BH: int, T: int, debug: bool = False):
    """Build the per-core Bass module.

    Inputs (per core):
      qt  [BH, 2D, T]    fp16  Q^T duplicated on both partition halves
      kt  [BH, 2D, T/2]  fp16  K^T: even k2-tiles rows 0:D, odd rows D:2D
      v   [BH, P, T/P, D] fp16 V with k2 split (tile, partition)
    Output:
      out [BH, D, T]   f32   out transposed (d-major)
    """
    assert T % 1024 == 0 and T % P == 0
    KT_TILES = T // P  # 16
    CH = 1024          # per-chunk q extent (2 PSUM banks)
    NPAIR = KT_TILES // 2
    schraud = _get_schraud_op()

    nc = bacc.Bacc("TRN2", target_bir_lowering=False, debug=debug)

    qt = nc.dram_tensor("qt", [BH, 2 * D, T], F16, kind="ExternalInput").ap()
    kt = nc.dram_tensor("kt", [BH, 2 * D, T // 2], F16, kind="ExternalInput").ap()
    v = nc.dram_tensor("v", [BH, P, KT_TILES, D], F16, kind="ExternalInput").ap()
    out = nc.dram_tensor("out", [BH, D, T], F32, kind="ExternalOutput").ap()

    with tile.TileContext(nc) as tc:
        with (
            tc.tile_pool(name="ins", bufs=1) as ins_pool,
            tc.tile_pool(name="et", bufs=14) as et_pool,
            tc.tile_pool(name="scr", bufs=2) as scr_pool,
            tc.tile_pool(name="small", bufs=20) as small_pool,
            tc.tile_pool(name="sall", bufs=3) as sall_pool,
            tc.tile_pool(name="vp", bufs=3) as vp_pool,
            tc.tile_pool(name="lvp", bufs=4) as lvp_pool,
            tc.tile_pool(name="osb", bufs=2) as osb_pool,
            tc.tile_pool(name="sps", bufs=3, space="PSUM") as sps_pool,
            tc.tile_pool(name="opsum", bufs=1, space="PSUM") as o_pool,
        ):
            qt_sb = ins_pool.tile([2 * D, BH, T], F16, tag="qt_sb")
            kt_sb = ins_pool.tile([2 * D, BH, T // 2], F16, tag="kt_sb")
            v_sb = ins_pool.tile([P, BH, KT_TILES, D], F16, tag="v_sb")
            c3sb = ins_pool.tile([P, 1], F32, tag="c3sb")
            nc.vector.memset(c3sb[:], SCH_C3)
            # Warm up the ACT exp table-set during the input DMAs.
            warm = small_pool.tile([P, 1], F32, tag="warm")
            nc.vector.memset(warm[:], 0.0)
            nc.scalar.activation(
                warm[:], warm[:], mybir.ActivationFunctionType.Exp
            )

            # Fine-grained input DMAs: first pair's work can start after
            # ~380KB instead of the full 4MB.
            for bh in range(BH):
                nc.sync.dma_start(qt_sb[:, bh, 0:T // 2], qt[bh][:, 0:T // 2])
                nc.sync.dma_start(kt_sb[:, bh, 0:T // 4],
                                  kt[bh][:, 0:T // 4])
                nc.sync.dma_start(qt_sb[:, bh, T // 2:T],
                                  qt[bh][:, T // 2:T])
                nc.sync.dma_start(kt_sb[:, bh, T // 4:T // 2],
                                  kt[bh][:, T // 4:T // 2])
                nc.sync.dma_start(v_sb[:, bh], v[bh])

            def emit_mm2(out_ps, vp, et, t):
                # col-group alternating order for PE column concurrency
                for cidx, c in enumerate((0, T // 2, 512, T // 2 + 512)):
                    half = c // (T // 2)
                    qh = c % (T // 2)
                    nc.tensor.matmul(
                        out_ps[half * D:(half + 1) * D, qh:qh + 512],
                        lhsT=vp[:],
                        rhs=et[:, c:c + 512],
                        start=(t == 0),
                        stop=(t == KT_TILES - 1),
                        skip_group_check=True,
                    )

            def evacuate(bh, out_ps):
                osb = osb_pool.tile([2 * D, T // 2], F32, tag="osb")
                if bh % 2 == 0:
                    nc.vector.tensor_copy(osb[:], out_ps[:])
                else:
                    nc.scalar.copy(osb[:], out_ps[:])
                nc.sync.dma_start(out[bh][:, 0:T // 2], osb[0:D])
                nc.sync.dma_start(out[bh][:, T // 2:T], osb[D:2 * D])

            def pop_mm2(pending):
                bh_, out_ps_, vp_, et_, t_ = pending.pop(0)
                emit_mm2(out_ps_, vp_, et_, t_)
                if t_ == KT_TILES - 1:
                    evacuate(bh_, out_ps_)

            HB = KT_TILES // 2  # tiles per half-bh batch (8)

            def finish_half(bh, half, s_all):
                """One reciprocal + one broadcast-multiply for 8 tiles."""
                rec = small_pool.tile([P, HB], F32, tag="rec")
                nc.vector.reciprocal_approx_fast(rec[:], s_all[:])
                vp_all = vp_pool.tile([P, HB, D], F16, tag="vp")
                rec_b = rec[:, :, None].broadcast_to([P, HB, D])
                nc.gpsimd.tensor_tensor(
                    vp_all[:],
                    v_sb[:, bh, half * HB:(half + 1) * HB, :],
                    rec_b,
                    mybir.AluOpType.mult,
                )
                return vp_all

            def finish_tile_now(bh, t, s_ap):
                """Per-tile normalization (used on the final half to overlap
                the mm2 tail with the remaining exps)."""
                rec = small_pool.tile([P, 1], F32, tag="rec1")
                nc.vector.reciprocal_approx_fast(rec[:], s_ap)
                vp = lvp_pool.tile([P, D], F16, tag="lvp")
                nc.vector.tensor_scalar_mul(vp[:], v_sb[:, bh, t, :], rec[:])
                return vp

            def exp_act(et_t, sp, q0):
                acc = small_pool.tile([P, 1], F32, tag="acc")
                nc.scalar.activation(
                    et_t[:, q0:q0 + CH],
                    sp[:],
                    mybir.ActivationFunctionType.Exp,
                    scale=SCALE,
                    accum_out=acc[:],
                )
                return acc

            def exp_dve(et_t, sp, q0):
                nc.vector._custom_dve(
                    schraud,
                    out=et_t[:, q0:q0 + CH].bitcast(I16),
                    in0=sp[:],
                    in1=c3sb[:],
                    s0=SCH_A,
                    s1=SCH_B,
                    imm2=SCH_M,
                )
                acc = small_pool.tile([P, 1], F32, tag="acc")
                sc = scr_pool.tile([P, CH // 2], F16, tag="scr")
                nc.vector.scalar_tensor_tensor(
                    sc[:],
                    et_t[:, q0:q0 + CH // 2],
                    1.0,
                    et_t[:, q0 + CH // 2:q0 + CH],
                    op0=mybir.AluOpType.mult,
                    op1=mybir.AluOpType.add,
                    accum_out=acc[:],
                )
                return acc

            pending_mm2 = []
            et_by_tile = {}
            for bh in range(BH):
                out_ps = o_pool.tile([2 * D, T // 2], F32, tag="out_ps")
                for half in range(2):
                    s_all = sall_pool.tile([P, HB], F32, tag="sall")
                    half_tiles = []
                    for jj in range(NPAIR // 2):
                        j = half * (NPAIR // 2) + jj
                        tA, tB = 2 * j, 2 * j + 1
                        lhsA = kt_sb[0:D, bh, j * P:(j + 1) * P]
                        lhsB = kt_sb[D:2 * D, bh, j * P:(j + 1) * P]
                        etA = et_pool.tile([P, T], F16, tag="et", name="etA")
                        etB = et_pool.tile([P, T], F16, tag="et", name="etB")
                        et_by_tile[(bh, tA)] = etA
                        et_by_tile[(bh, tB)] = etB
                        accs_A, accs_B = [], []
                        for c in range(2):
                            q0 = c * CH
                            spA = sps_pool.tile([P, CH], F32, tag="sp",
                                                name="spA")
                            spB = sps_pool.tile([P, CH], F32, tag="sp",
                                                name="spB")
                            for cc in (0, 512):
                                nc.tensor.matmul(
                                    spA[:, cc:cc + 512],
                                    lhsT=lhsA,
                                    rhs=qt_sb[0:D, bh, q0 + cc:q0 + cc + 512],
                                    start=True,
                                    stop=True,
                                )
                                nc.tensor.matmul(
                                    spB[:, cc:cc + 512],
                                    lhsT=lhsB,
                                    rhs=qt_sb[D:2 * D, bh,
                                              q0 + cc:q0 + cc + 512],
                                    start=True,
                                    stop=True,
                                )
                            accs_A.append(exp_act(etA, spA, q0))
                            if (j, c) in ACT_EXTRA:
                                accs_B.append(exp_act(etB, spB, q0))
                            else:
                                accs_B.append(exp_dve(etB, spB, q0))
                        last_half = (bh == BH - 1 and half == 1)
                        # chunk-sum merges land in this half's s_all slice
                        for t, accs in ((tA, accs_A), (tB, accs_B)):
                            k = t - half * HB
                            nc.gpsimd.tensor_tensor(
                                s_all[:, k:k + 1], accs[0][:], accs[1][:],
                                mybir.AluOpType.add,
                            )
                            if last_half:
                                vp = finish_tile_now(bh, t, s_all[:, k:k + 1])
                                pending_mm2.append(
                                    (bh, out_ps, vp, et_by_tile.pop((bh, t)), t)
                                )
                            else:
                                half_tiles.append(t)
                        # drain two mm2 tiles from the previous half per pair
                        if last_half:
                            while len(pending_mm2) > 2:
                                pop_mm2(pending_mm2)
                        else:
                            for _ in range(2):
                                if pending_mm2:
                                    pop_mm2(pending_mm2)
                    if not (bh == BH - 1 and half == 1):
                        vp_all = finish_half(bh, half, s_all)
                        for t in half_tiles:
                            et_t = et_by_tile.pop((bh, t))
                            pending_mm2.append(
                                (bh, out_ps, vp_all[:, t - half * HB, :],
                                 et_t, t)
                            )
            while pending_mm2:
                pop_mm2(pending_mm2)

    nc.compile()
    return nc


_NC_CACHE: dict = {}

# Debug/profiling knobs (used by the local test harness; harmless defaults).
TRACE = False
LAST_RESULTS = None


def _get_nc(BH: int, T: int):
    key = (BH, T)
    if key not in _NC_CACHE:
        _NC_CACHE[key] = build_attention_nc(BH, T)
    return _NC_CACHE[key]


def _reference_numpy(Q, K, V, padding_mask, isCausal):
    """Fallback exactly mirroring reference.py (never hit for spec inputs)."""
    Q = Q.astype(np.float64)
    K = K.astype(np.float64)
    V = V.astype(np.float64)
    scores = np.einsum("bhqd,bhkd->bhqk", Q, K) * SCALE
    T1 = scores.shape[2]
    mask = padding_mask[:, None, :, :].astype(np.float64)
    if isCausal:
        mask = mask * np.tril(np.ones((T1, T1)))
    scores = np.where(mask == 0, -np.inf, scores)
    m = np.max(scores, axis=2, keepdims=True)
    e = np.exp(scores - m)
    attn = e / np.sum(e, axis=2, keepdims=True)
    return np.einsum("bhqk,bhkd->bhqd", attn, V).astype(np.float32)


def kernel(Q, K, V, padding_mask, isCausal, **_unused):
    Q = np.asarray(Q, dtype=np.float32)
    K = np.asarray(K, dtype=np.float32)
    V = np.asarray(V, dtype=np.float32)
    padding_mask = np.asarray(padding_mask)
    causal = int(np.asarray(isCausal))

    B, H, T, Dd = Q.shape
    assert Dd == D
    if causal != 0 or padding_mask.min() != 1.0 or padding_mask.max() != 1.0:
        return _reference_numpy(Q, K, V, padding_mask, causal)

    BHT = B * H
    assert BHT % N_CORES == 0
    BH = BHT // N_CORES  # pairs per core

    nc = _get_nc(BH, T)

    # Host-side layout prep (contiguous per-core shards).
    Qf = Q.reshape(BHT, T, D)
    Kf = K.reshape(BHT, T, D)
    Vf = V.reshape(BHT, T, D)

    QT = Qf.transpose(0, 2, 1).astype(np.float16)  # [BHT, D, T]
    qt_all = np.ascontiguousarray(
        np.concatenate([QT, QT], axis=1)
    )  # [BHT, 2D, T] fp16, duplicated across partition halves
    KT = Kf.transpose(0, 2, 1).astype(np.float16)  # [BHT, D, T]
    KT4 = KT.reshape(BHT, D, T // 128, 128)
    kt_all = np.ascontiguousarray(
        np.concatenate(
            [
                KT4[:, :, 0::2, :].reshape(BHT, D, T // 2),
                KT4[:, :, 1::2, :].reshape(BHT, D, T // 2),
            ],
            axis=1,
        )
    )  # [BHT, 2D, T/2] fp16: even k2-tiles top, odd bottom
    # V -> [BHT, P, T/P, D]: v_dev[b, p, t, d] = V[b, t*128 + p, d]
    v_all = np.ascontiguousarray(
        Vf.reshape(BHT, T // P, P, D).transpose(0, 2, 1, 3).astype(np.float16)
    )

    in_maps = []
    for c in range(N_CORES):
        sl = slice(c * BH, (c + 1) * BH)
        in_maps.append(
            {
                "qt": np.ascontiguousarray(qt_all[sl]),
                "kt": np.ascontiguousarray(kt_all[sl]),
                "v": np.ascontiguousarray(v_all[sl]),
            }
        )

    res = None
    last_err = None
    for attempt in range(3):
        try:
            res = run_bass_kernel_spmd(
                nc, in_maps, core_ids=list(range(N_CORES)), trace=TRACE
            )
            break
        except Exception as e:  # transient device/runtime hiccup -> retry
            last_err = e
            import time as _time

            _time.sleep(2.0)
    if res is None:
        raise last_err
    global LAST_RESULTS
    LAST_RESULTS = res

    # Gather: each core returns out [BH, D, T] -> [BHT, T, D] -> [B, H, T, D]
    outs = [res.results[c]["out"] for c in range(N_CORES)]
    out_all = np.concatenate(outs, axis=0)  # [BHT, D, T]
    out = out_all.transpose(0, 2, 1).reshape(B, H, T, D)
    return np.ascontiguousarray(out).astype(np.float32)
